# revision 1
# baseline (speedup 1.0000x reference)
"""GATv2Conv(64, 1024, heads=16) + Linear(16384, 20) Trainium2 kernel.

Strategy: shard by DESTINATION node (512 nodes/core on 8 cores). Host
pre-gathers per-edge x rows and builds 0/1 segment matrices, so the device
kernel is 100% regular dense ops (no indirect addressing):

  per node-tile (128 dst nodes, <=768 incoming edges padded, 6 subtiles):
    Y    = [x_src | x_dst] @ [W_l ; W_r]        (PE, K=128, bf16)
    lY   = leaky_relu(Y)                        (Act, PSUM->SBUF)
    t    = sum_c att_c * lY_c                   (DVE fused mult+reduce)
    L    = per-head logits; P = exp(L)          (no max-sub needed: |L|<~8)
    segsum = S01T.T @ P ; denom = S01.T @ segsum  (PE does the segment
                                                 sum AND the broadcast-back)
    alpha  = P * 1/denom                        (DVE)
    G_h^T  = (alpha_h * X_src).T @ S01T         (PE, 64x128 per head)
    aggT   = W_l_chunk.T @ G_h^T                (PE; assoc. trick:
                                                 (Salpha@X)@W == S@(X@W))
    reluT  = relu(aggT + bias)                  (Act, bias per-partition)
    z      = sum_chunks reluT_chunk.T @ W_out_chunk   (PE)

Output z slices are concatenated on host; b_out added on host.
"""

import numpy as np

N_NODES = 4096
N_EDGES = 16384
F_IN = 64
H = 16
C = 1024
HC = H * C  # 16384
N_CLASS = 20
N_CORES = 8
NODES_PER_CORE = N_NODES // N_CORES  # 512
TILES_PER_CORE = 4  # node-tiles of 128 dst nodes
NT = 128  # nodes per tile
EPT = 768  # padded edges per node-tile (6 subtiles of 128)
SUB = EPT // 128  # 6
NSUB = TILES_PER_CORE * SUB  # 24 subtiles per core
EPC = TILES_PER_CORE * EPT  # 3072 padded edges per core
YCHUNK = 512  # Y matmul free-dim chunk (1 PSUM bank)
NCH = HC // YCHUNK  # 32 chunks
NCK = HC // 128  # 128 channel chunks for agg/z
NEG_SLOPE = 0.2

_CACHE = {}


def _build_nc():
    import concourse.bacc as bacc
    import concourse.bass as bass
    import concourse.mybir as mybir
    import concourse.tile as tile

    f32 = mybir.dt.float32
    bf16 = mybir.dt.bfloat16
    AF = mybir.ActivationFunctionType
    OP = mybir.AluOpType

    nc = bacc.Bacc("TRN2", target_bir_lowering=False)

    # -------- dram I/O --------
    d_xcatT = nc.dram_tensor("xcatT", [128, EPC], bf16, kind="ExternalInput")
    d_Wcat = nc.dram_tensor("Wcat", [128, HC], bf16, kind="ExternalInput")
    d_att = nc.dram_tensor("att_rep", [128, HC], bf16, kind="ExternalInput")
    d_S01T = nc.dram_tensor("S01T", [128, NSUB, NT], bf16, kind="ExternalInput")
    d_S01 = nc.dram_tensor("S01", [128, EPC], bf16, kind="ExternalInput")
    d_Xsrc = nc.dram_tensor("Xsrc", [128, NSUB, F_IN], bf16, kind="ExternalInput")
    d_Wout = nc.dram_tensor("Wout", [128, NCK, N_CLASS], bf16, kind="ExternalInput")
    d_bias = nc.dram_tensor("bias", [128, NCK], f32, kind="ExternalInput")
    d_base = nc.dram_tensor("base", [128, NSUB * H], f32, kind="ExternalInput")
    d_z = nc.dram_tensor("z", [128, TILES_PER_CORE, N_CLASS], f32, kind="ExternalOutput")

    with tile.TileContext(nc) as tc:
        with (
            tc.tile_pool(name="const", bufs=1) as cpool,
            tc.tile_pool(name="ly", bufs=3) as lypool,
            tc.tile_pool(name="scr", bufs=3) as scrpool,
            tc.tile_pool(name="small", bufs=2) as smpool,
            tc.tile_pool(name="xw", bufs=3) as xwpool,
            tc.tile_pool(name="gsb", bufs=2) as gpool,
            tc.tile_pool(name="relu", bufs=2) as rpool,
            tc.tile_pool(name="zout", bufs=1) as zpool,
            tc.tile_pool(name="psY", bufs=2, space=bass.MemorySpace.PSUM) as psY,
            tc.tile_pool(name="psSS", bufs=1, space=bass.MemorySpace.PSUM) as psSS,
            tc.tile_pool(name="psDen", bufs=1, space=bass.MemorySpace.PSUM) as psDen,
            tc.tile_pool(name="psG", bufs=1, space=bass.MemorySpace.PSUM) as psG,
            tc.tile_pool(name="psA", bufs=2, space=bass.MemorySpace.PSUM) as psA,
            tc.tile_pool(name="psZ", bufs=1, space=bass.MemorySpace.PSUM) as psZ,
        ):
            # -------- load constants --------
            xcatT = cpool.tile([128, EPC], bf16)
            Wcat = cpool.tile([128, HC], bf16)
            attr = cpool.tile([128, HC], bf16)
            S01T = cpool.tile([128, NSUB, NT], bf16)
            S01 = cpool.tile([128, EPC], bf16)
            Xsrc = cpool.tile([128, NSUB, F_IN], bf16)
            Wout = cpool.tile([128, NCK, N_CLASS], bf16)
            bias = cpool.tile([128, NCK], f32)
            base = cpool.tile([128, NSUB * H], f32)
            nc.sync.dma_start(base[:], d_base[:])
            nc.sync.dma_start(xcatT[:], d_xcatT[:])
            nc.sync.dma_start(Wcat[:], d_Wcat[:])
            nc.sync.dma_start(attr[:], d_att[:])
            nc.sync.dma_start(S01T[:], d_S01T[:])
            nc.sync.dma_start(S01[:], d_S01[:])
            nc.sync.dma_start(Xsrc[:], d_Xsrc[:])
            nc.sync.dma_start(Wout[:], d_Wout[:])
            nc.sync.dma_start(bias[:], d_bias[:])

            z_sb = zpool.tile([128, TILES_PER_CORE, N_CLASS], f32)

            def phase_Y(t, tch):
                """Per-edge logit contributions for node-tile t."""
                for s in range(SUB):
                    col0 = t * EPT + s * 128
                    for k in range(NCH):
                        y = psY.tile([128, YCHUNK], f32, tag="y")
                        nc.tensor.matmul(
                            y[:],
                            xcatT[:, col0 : col0 + 128],
                            Wcat[:, k * YCHUNK : (k + 1) * YCHUNK],
                        )
                        ly = lypool.tile([128, YCHUNK], bf16, tag="ly")
                        nc.scalar.activation(ly[:], y[:], AF.Abs)
                        scr = scrpool.tile([128, YCHUNK], bf16, tag="scr")
                        nc.vector.tensor_tensor(
                            out=scr[:],
                            in0=ly[:],
                            in1=attr[:, k * YCHUNK : (k + 1) * YCHUNK],
                            op=OP.mult,
                        )
                        col = (k % 2) * (SUB * H) + s * H + (k // 2)
                        nc.vector.tensor_reduce(
                            out=tch[:, col : col + 1],
                            in_=scr[:],
                            axis=mybir.AxisListType.X,
                            op=OP.add,
                        )

            def phase_rest(t, tch):
                """Softmax + aggregation + output for node-tile t."""
                # L[e, s*H+h] = tch halves + 0.6*(sl[src]+sr[dst]) host-baked base
                SH = SUB * H
                L = smpool.tile([128, SH], f32, tag="L")
                nc.vector.tensor_tensor(
                    out=L[:], in0=tch[:, 0:SH], in1=tch[:, SH : 2 * SH], op=OP.add
                )
                L2 = smpool.tile([128, SH], f32, tag="L2")
                nc.vector.tensor_tensor(
                    out=L2[:],
                    in0=L[:],
                    in1=base[:, t * SH : (t + 1) * SH],
                    op=OP.add,
                )
                P = smpool.tile([128, SH], bf16, tag="P")
                nc.scalar.activation(P[:], L2[:], AF.Exp)

                # segment sums over incoming edges (PE): [128n, 16]
                ss = psSS.tile([128, H], f32, tag="ss")
                for s in range(SUB):
                    nc.tensor.matmul(
                        ss[:],
                        S01T[:, t * SUB + s, :],
                        P[:, s * H : (s + 1) * H],
                        start=(s == 0),
                        stop=(s == SUB - 1),
                    )
                ssb = smpool.tile([128, H], bf16, tag="ssb")
                nc.scalar.copy(ssb[:], ss[:])

                # broadcast denom back to edges (PE): [128e, 6*16]
                den = psDen.tile([128, SUB * H], f32, tag="den")
                for s in range(SUB):
                    e0 = t * EPT + s * 128
                    nc.tensor.matmul(
                        den[:, s * H : (s + 1) * H],
                        S01[:, e0 : e0 + 128],
                        ssb[:],
                    )
                rec = smpool.tile([128, SUB * H], f32, tag="rec")
                nc.vector.reciprocal(rec[:], den[:])
                alpha = smpool.tile([128, SUB * H], f32, tag="alpha")
                nc.vector.tensor_tensor(
                    out=alpha[:], in0=P[:], in1=rec[:], op=OP.mult
                )

                # G_h^T = (alpha_h * X_src).T @ S01T  : [64, 128] per head
                G = gpool.tile([64, H, NT], bf16, tag="G")
                for h in range(H):
                    g = psG.tile([64, NT], f32, tag="g")
                    for s in range(SUB):
                        xw = xwpool.tile([128, F_IN], bf16, tag="xw")
                        nc.vector.tensor_scalar_mul(
                            xw[:],
                            Xsrc[:, t * SUB + s, :],
                            alpha[:, s * H + h : s * H + h + 1],
                        )
                        nc.tensor.matmul(
                            g[:],
                            xw[:],
                            S01T[:, t * SUB + s, :],
                            start=(s == 0),
                            stop=(s == SUB - 1),
                        )
                    nc.scalar.copy(G[:, h, :], g[:])

                # aggT chunks + bias + relu, then final linear into psZ
                zp = psZ.tile([128, N_CLASS], f32, tag="zp")
                reluT = rpool.tile([128, NCK, NT], bf16, tag="reluT")
                for kk in range(NCK):
                    h = kk // 8
                    a = psA.tile([128, NT], f32, tag="a")
                    nc.tensor.matmul(
                        a[:],
                        Wcat[0:64, kk * 128 : (kk + 1) * 128],
                        G[:, h, :],
                    )
                    nc.scalar.activation(
                        reluT[:, kk, :], a[:], AF.Relu, bias=bias[:, kk : kk + 1]
                    )
                for kk in range(NCK):
                    nc.tensor.matmul(
                        zp[:],
                        reluT[:, kk, :],
                        Wout[:, kk, :],
                        start=(kk == 0),
                        stop=(kk == NCK - 1),
                    )
                nc.vector.tensor_copy(z_sb[:, t, :], zp[:])

            # software-pipelined emission: Y(t+1) is issued before rest(t)
            tchs = []
            for t in range(TILES_PER_CORE):
                tch = smpool.tile([128, 2 * SUB * H], f32, tag=f"tch{t}")
                tchs.append(tch)
                phase_Y(t, tch)
                if t > 0:
                    phase_rest(t - 1, tchs[t - 1])
            phase_rest(TILES_PER_CORE - 1, tchs[TILES_PER_CORE - 1])

            nc.sync.dma_start(d_z[:], z_sb[:])

    nc.compile()
    return nc


def _prep_inputs(x, edge_index, W_l, W_r, att, bias_gat, W_out, b_out):
    import ml_dtypes

    bf16 = ml_dtypes.bfloat16
    x = np.asarray(x, np.float32)
    W_l = np.asarray(W_l, np.float32)
    W_r = np.asarray(W_r, np.float32)
    att = np.asarray(att, np.float32)
    bias_gat = np.asarray(bias_gat, np.float32)
    W_out = np.asarray(W_out, np.float32)

    src = np.concatenate([np.asarray(edge_index[0]), np.arange(N_NODES)]).astype(
        np.int64
    )
    dst = np.concatenate([np.asarray(edge_index[1]), np.arange(N_NODES)]).astype(
        np.int64
    )

    # shared (replicated) tensors
    Wcat = np.vstack([W_l, W_r]).astype(bf16)  # [128, HC]
    att_rep = np.broadcast_to(0.4 * att.reshape(1, HC), (128, HC)).astype(bf16)
    Wout_sb = np.ascontiguousarray(
        W_out.reshape(NCK, 128, N_CLASS).transpose(1, 0, 2)
    ).astype(bf16)  # [p, k, 20] = W_out[k*128+p]
    bias_sb = np.ascontiguousarray(bias_gat.reshape(NCK, 128).T).astype(
        np.float32
    )  # [p, k]

    # per-node logit scalars: sl[n,h] = att[h] . (x[n] @ W_l)[h]
    ul = np.einsum("fhc,hc->fh", W_l.reshape(F_IN, H, C), att)
    ur = np.einsum("fhc,hc->fh", W_r.reshape(F_IN, H, C), att)
    sl = x @ ul  # [N, H]
    sr = x @ ur

    in_maps = []
    order = np.argsort(dst, kind="stable")
    src_s, dst_s = src[order], dst[order]
    tile_of = dst_s // NT  # global node-tile id 0..31
    for core in range(N_CORES):
        xcatT = np.zeros((128, EPC), np.float32)
        Xsrc = np.zeros((128, NSUB, F_IN), np.float32)
        S01T = np.zeros((128, NSUB, NT), np.float32)
        base = np.zeros((128, NSUB, H), np.float32)
        for t in range(TILES_PER_CORE):
            gt = core * TILES_PER_CORE + t
            idx = np.nonzero(tile_of == gt)[0]
            ne = len(idx)
            assert ne <= EPT, f"node-tile {gt} has {ne} edges > {EPT}"
            es, ed = src_s[idx], dst_s[idx]
            slot = np.arange(ne)
            s_sub, p = slot // 128 + t * SUB, slot % 128
            xcatT[0:64, t * EPT + slot] = x[es].T
            xcatT[64:128, t * EPT + slot] = x[ed].T
            Xsrc[p, s_sub, :] = x[es]
            S01T[p, s_sub, ed - gt * NT] = 1.0
            base[p, s_sub, :] = 0.6 * (sl[es] + sr[ed])
        S01 = np.ascontiguousarray(
            S01T.transpose(2, 1, 0).reshape(NT, NSUB * 128)
        )  # [n, subtile*128+p] -- must match edge column order t*EPT+s*128+p
        # pad slots: point their denominator at node 0 so 1/denom stays finite
        # (their S01T rows stay zero, so they contribute nothing downstream)
        pad_mask = S01.sum(axis=0) == 0.0
        S01[0, pad_mask] = 1.0
        in_maps.append(
            {
                "xcatT": xcatT.astype(bf16),
                "Wcat": Wcat,
                "att_rep": att_rep,
                "S01T": S01T.astype(bf16),
                "S01": S01.astype(bf16),
                "Xsrc": Xsrc.astype(bf16),
                "Wout": Wout_sb,
                "bias": bias_sb,
                "base": base.reshape(128, NSUB * H),
            }
        )
    return in_maps


def kernel(**inputs):
    from concourse.bass_utils import run_bass_kernel_spmd

    if "nc" not in _CACHE:
        _CACHE["nc"] = _build_nc()
    nc = _CACHE["nc"]

    in_maps = _prep_inputs(**inputs)
    res = run_bass_kernel_spmd(nc, in_maps, list(range(N_CORES)))
    b_out = np.asarray(inputs["b_out"], np.float32)
    z = np.empty((N_NODES, N_CLASS), np.float32)
    for core in range(N_CORES):
        zc = np.asarray(res.results[core]["z"], np.float32)  # [128, 4, 20]
        for t in range(TILES_PER_CORE):
            n0 = core * NODES_PER_CORE + t * NT
            z[n0 : n0 + NT] = zc[:, t, :]
    return z + b_out

